# revision 1
# baseline (speedup 1.0000x reference)
"""Trainium2 Bass kernel for nn_CrossAttention (B=2,N=6,D=128,M=625,H=28,W=60, 4 heads x 32).

Sharding: 8 cores = 2 batches x 4 query-token shards. Zero collectives.
Each core computes full cross-attention + MLP for its query shard.
"""

import numpy as np

import concourse.bass as bass
import concourse.mybir as mybir
import concourse.tile as tile
from concourse import bass_utils
from concourse.vector_clock import ScopedClock, VectorClock
from concourse.tile_scheduler import N_PROCS

F32 = mybir.dt.float32
BF16 = mybir.dt.bfloat16
AF = mybir.ActivationFunctionType
OP = mybir.AluOpType

B, N, D, M, H, W = 2, 6, 128, 625, 28, 60
HEADS, DHEAD = 4, 32
NQ_FULL = N * M            # 3750
NK = N * H * W             # 10080
TQ = 938                   # padded per-core query shard
EPS = 1e-5

KT = 128                   # k/v & token tile size
N_KT = (NK + KT - 1) // KT          # 79 (last = 96)
N_QT = (TQ + KT - 1) // KT          # 8  (last = 42)
Q_CHUNKS = [(0, 512), (512, TQ - 512)]   # attention q chunks


def _split_multiwait_json(bir_json: bytes) -> bytes:
    """This walrus build allows only one sync-wait per instruction: move
    extra on_wait entries onto EventSemaphore instructions inserted just
    before the owner (same engine, so ordering is preserved)."""
    import json
    bir = json.loads(bir_json)
    n_fix = 0
    for fn in bir["functions"]:
        for blk in fn["blocks"]:
            out = []
            for ins in blk["instructions"]:
                si = ins.get("sync_info")
                waits = (si or {}).get("on_wait") or []
                if len(waits) > 1:
                    for wi, w in enumerate(waits[:-1]):
                        out.append({
                            "debug": ins.get("debug", 0),
                            "engine": ins["engine"],
                            "ins": [], "outs": [],
                            "name": f"{ins['name']}-xw{wi}",
                            "opcode": "EventSemaphore",
                            "sync_info": {"on_update": [], "on_wait": [w]},
                        })
                        n_fix += 1
                    si["on_wait"] = [waits[-1]]
                out.append(ins)
            blk["instructions"] = out
    return json.dumps(bir).encode()


def _install_compile_patch():
    from concourse import bass_utils as bu
    if getattr(bu, "_mw_patched", False):
        return
    orig = bu.compile_bir_kernel

    def patched(bir_json, tmpdir, neff_name="file.neff"):
        return orig(_split_multiwait_json(bir_json), tmpdir, neff_name)

    bu.compile_bir_kernel = patched
    bu._mw_patched = True
    try:
        from concourse import bass2jax
        if getattr(bass2jax, "compile_bir_kernel", None) is orig:
            bass2jax.compile_bir_kernel = patched
    except ImportError:
        pass


class _SplitDrainTileContext(tile.TileContext):
    """This walrus build rejects >1 sem wait on a Drain; split the exit
    drain's waits across per-proc drains (one wait each)."""

    def _drain_and_barrier(self, tick_clock, wait_clock):
        full = tick_clock.global_clock
        for p in range(N_PROCS):
            mask = VectorClock([(1 << 30) if i == p else 0 for i in range(N_PROCS)])
            partial = full.copy()
            partial.elementwise_min(mask)
            d = self.nc.sync.drain()
            wait_clock.add_sem_waits(d.ins, ScopedClock({None: partial}))
        self.nc.all_engine_barrier()
        assert self.sems is not None
        popped = self.nc._tile_sem_poison_stack.pop()
        assert popped is self._sem_poison
        self.nc.clear_and_free_semaphores(list(self.sems.allocated().values()))
        self.nc.all_engine_barrier()


def _ln_proj_phase(nc, tc, ctx_pools, x_sb, n_tok, w_sb, csum_sb, bias_sb,
                   dst_mode, dst, onesc, identity, eps_ap):
    """LayerNorm + projection for one tensor, feature-major input x_sb [128, n_tok].

    dst_mode: 'vpack'  -> dst [128, n_tiles*132] token-major packed (32 cols/head + ones col)
              'featT'  -> dst [128, n_tok] feature-major (PE-transposed)
    """
    import contextlib
    n_tiles = (n_tok + KT - 1) // KT
    with contextlib.ExitStack() as ctx:
        sp = ctx.enter_context(tc.tile_pool(name="stat_ps", bufs=1, space="PSUM"))
        wp = ctx.enter_context(tc.tile_pool(name="lnp_work", bufs=3))
        gp = ctx.enter_context(tc.tile_pool(name="lnp_g", bufs=3, space="PSUM"))
        x2p = ctx.enter_context(tc.tile_pool(name="lnp_x2", bufs=2))
        stp = ctx.enter_context(tc.tile_pool(name="lnp_stats", bufs=1))
        trp = ctx.enter_context(tc.tile_pool(name="lnp_tr", bufs=3, space="PSUM"))

        # ---- pass 1: per-token sum & sumsq via PE (x / x^2 stationary, ones rhs)
        spsum = sp.tile([128, 2 * n_tiles], F32)
        CH = 1024
        ti = 0
        for off in range(0, n_tok, CH):
            csz = min(CH, n_tok - off)
            x2 = x2p.tile([128, CH], F32, tag="x2")
            nc.gpsimd.tensor_mul(x2[:, :csz], x_sb[:, off:off + csz],
                                 x_sb[:, off:off + csz])
            for i in range(0, csz, KT):
                tsz = min(KT, csz - i)
                nc.tensor.matmul(spsum[0:tsz, 2 * ti:2 * ti + 1],
                                 x_sb[:, off + i:off + i + tsz],
                                 onesc[:, 0:1], start=True, stop=True)
                nc.tensor.matmul(spsum[0:tsz, 2 * ti + 1:2 * ti + 2],
                                 x2[:, i:i + tsz],
                                 onesc[:, 0:1], start=True, stop=True)
                ti += 1
        assert ti == n_tiles

        # ---- batched stats math: negmu, alpha (=rsqrt(var+eps)) per token tile col
        nmu = stp.tile([128, n_tiles], F32, tag="nmu")
        alpha = stp.tile([128, n_tiles], F32, tag="alpha")
        msq = wp.tile([128, n_tiles], F32, tag="msq")
        mu2 = wp.tile([128, n_tiles], F32, tag="mu2")
        var = wp.tile([128, n_tiles], F32, tag="var")
        sd = wp.tile([128, n_tiles], F32, tag="sd")
        sev = spsum[:, :].rearrange("p (t two) -> p t two", two=2)
        nc.vector.tensor_scalar(nmu[:, :], sev[:, :, 0:1].rearrange("p t o -> p (t o)"),
                                -1.0 / D, None, OP.mult)
        nc.vector.tensor_scalar(msq[:, :], sev[:, :, 1:2].rearrange("p t o -> p (t o)"),
                                1.0 / D, None, OP.mult)
        nc.vector.tensor_mul(mu2[:, :], nmu[:, :], nmu[:, :])
        nc.vector.tensor_sub(var[:, :], msq[:, :], mu2[:, :])
        nc.scalar.activation(sd[:, :], var[:, :], AF.Sqrt, bias=eps_ap)
        nc.vector.reciprocal(alpha[:, :], sd[:, :])

        # ---- pass 2: projection + LN-apply (+ optional transpose)
        for i in range(n_tiles):
            off = i * KT
            tsz = min(KT, n_tok - off)
            g = gp.tile([128, 128], F32, tag="g")
            nc.tensor.matmul(g[0:tsz, :], x_sb[:, off:off + tsz], w_sb[:, :],
                             start=True, stop=True)
            t1 = wp.tile([128, 128], F32, tag="t1")
            # t1 = (csum_bcast * negmu) + G
            nc.vector.scalar_tensor_tensor(
                t1[0:tsz, :], csum_sb[0:tsz, :], nmu[0:tsz, i:i + 1], g[0:tsz, :],
                op0=OP.mult, op1=OP.add)
            if dst_mode == "vpack":
                dv = dst[:, 132 * i:132 * i + 132].rearrange(
                    "p (h c) -> p h c", h=HEADS)[:, :, 0:DHEAD]
                nc.vector.scalar_tensor_tensor(
                    dv[0:tsz, :, :],
                    t1[0:tsz, :].rearrange("p (h c) -> p h c", c=DHEAD),
                    alpha[0:tsz, i:i + 1],
                    bias_sb[0:tsz, :].rearrange("p (h c) -> p h c", c=DHEAD),
                    op0=OP.mult, op1=OP.add)
            else:
                ap = wp.tile([128, 128], F32, tag="applied")
                nc.vector.scalar_tensor_tensor(
                    ap[0:tsz, :], t1[0:tsz, :], alpha[0:tsz, i:i + 1],
                    bias_sb[0:tsz, :], op0=OP.mult, op1=OP.add)
                tr = trp.tile([128, 128], F32, tag="tr")
                nc.tensor.matmul(tr[:, 0:tsz], ap[0:tsz, :],
                                 identity[0:tsz, 0:tsz], is_transpose=True,
                                 start=True, stop=True)
                nc.vector.tensor_copy(dst[:, off:off + tsz], tr[:, 0:tsz])


def build_program(host):
    nc = bass.Bass()

    def inp(name, shape):
        return nc.dram_tensor(name, list(shape), F32, kind="ExternalInput")

    xq = inp("xq", (128, TQ))
    xk = inp("xk", (128, NK))
    xv = inp("xv", (128, NK))
    xskip = inp("xskip", (128, TQ))
    wq = inp("wq", (128, 128))
    wk = inp("wk", (128, 128))
    wv = inp("wv", (128, 128))
    csq = inp("csq", (128, 128))
    csk = inp("csk", (128, 128))
    csv = inp("csv", (128, 128))
    bqb = inp("bqb", (128, 128))
    bkb = inp("bkb", (128, 128))
    bvb = inp("bvb", (128, 128))
    identity = inp("identity", (128, 128))
    onesc = inp("onesc", (128, 128))
    pjA = inp("pjA", (128, 128))
    pjB = inp("pjB", (128, 128))
    w1 = inp("w1", (128, 2 * D))
    w2a = inp("w2a", (128, 128))
    w2b = inp("w2b", (128, 128))
    pbrow = inp("pbrow", (1, 128))
    b1row = inp("b1row", (1, 2 * D))
    b2row = inp("b2row", (1, 128))
    y = nc.dram_tensor("y", [128, TQ], F32, kind="ExternalOutput")

    has_proj_b = host["has_proj_b"]
    has_b1 = host["has_b1"]
    has_b2 = host["has_b2"]
    has_post = host["has_post"]
    if has_post:
        pogb = inp("pogb", (128, 128))
        pobb = inp("pobb", (128, 128))

    with _SplitDrainTileContext(nc) as tc:
        import contextlib
        with contextlib.ExitStack() as ctx:
            cpool = ctx.enter_context(tc.tile_pool(name="consts", bufs=1))
            big = ctx.enter_context(tc.tile_pool(name="big", bufs=1))

            def load_const(t, shape):
                s = cpool.tile(list(shape), F32, tag=t.name)
                nc.sync.dma_start(out=s[:], in_=t[:])
                return s

            wq_s = load_const(wq, (128, 128))
            wk_s = load_const(wk, (128, 128))
            wv_s = load_const(wv, (128, 128))
            csq_s = load_const(csq, (128, 128))
            csk_s = load_const(csk, (128, 128))
            csv_s = load_const(csv, (128, 128))
            bqb_s = load_const(bqb, (128, 128))
            bkb_s = load_const(bkb, (128, 128))
            bvb_s = load_const(bvb, (128, 128))
            id_s = load_const(identity, (128, 128))
            ones_s = load_const(onesc, (128, 128))
            pjA_s = load_const(pjA, (128, 128))
            pjB_s = load_const(pjB, (128, 128))
            w1_s = load_const(w1, (128, 2 * D))
            w2a_s = load_const(w2a, (128, 128))
            w2b_s = load_const(w2b, (128, 128))
            pb_s = load_const(pbrow, (1, 128)) if has_proj_b else None
            b1_s = load_const(b1row, (1, 2 * D)) if has_b1 else None
            b2_s = load_const(b2row, (1, 128)) if has_b2 else None
            if has_post:
                pog_s = load_const(pogb, (128, 128))
                pob_s = load_const(pobb, (128, 128))

            eps_s = cpool.tile([128, 1], F32, tag="eps")
            nc.vector.memset(eps_s[:, :], EPS)
            vpack = big.tile([128, N_KT * 132], F32, tag="vpack")
            khT = big.tile([128, NK], BF16, tag="khT")
            qhT = big.tile([128, TQ], BF16, tag="qhT")
            aTA = big.tile([128, TQ], F32, tag="aTA")
            aTB = big.tile([128, TQ], F32, tag="aTB")
            z_sb = big.tile([128, N_QT * 128], F32, tag="z_sb")
            outfm = big.tile([128, TQ], F32, tag="outfm")

            # ---------------- front: LN + projections ----------------
            # ones columns of vpack (Z trick), junk rows of aT (zero-killed by pjA/pjB)
            nc.vector.memset(
                vpack[:, :].rearrange("p (t h c) -> p t h c", h=HEADS, c=33)[:, :, :, 32:33],
                1.0)
            nc.vector.memset(aTA[:, :], 0.0)
            nc.vector.memset(aTB[:, :], 0.0)

            with tc.tile_pool(name="xv_pool", bufs=1) as xvp:
                xv_sb = xvp.tile([128, NK], F32)
                nc.sync.dma_start(out=xv_sb[:], in_=xv[:])
                _ln_proj_phase(nc, tc, None, xv_sb, NK, wv_s, csv_s, bvb_s,
                               "vpack", vpack, ones_s, id_s, eps_s[:, 0:1])
            with tc.tile_pool(name="xk_pool", bufs=1) as xkp:
                xk_sb = xkp.tile([128, NK], F32)
                nc.sync.dma_start(out=xk_sb[:], in_=xk[:])
                _ln_proj_phase(nc, tc, None, xk_sb, NK, wk_s, csk_s, bkb_s,
                               "featT", khT, ones_s, id_s, eps_s[:, 0:1])
            with tc.tile_pool(name="xq_pool", bufs=1) as xqp:
                xq_sb = xqp.tile([128, TQ], F32)
                nc.sync.dma_start(out=xq_sb[:], in_=xq[:])
                _ln_proj_phase(nc, tc, None, xq_sb, TQ, wq_s, csq_s, bqb_s,
                               "featT", qhT, ones_s, id_s, eps_s[:, 0:1])

            # ---------------- attention ----------------
            with contextlib.ExitStack() as actx:
                scp = actx.enter_context(tc.tile_pool(name="sc_ps", bufs=1, space="PSUM"))
                avp = actx.enter_context(tc.tile_pool(name="av_ps", bufs=1, space="PSUM"))
                pep = actx.enter_context(tc.tile_pool(name="pexp", bufs=3))
                zrp = actx.enter_context(tc.tile_pool(name="zr", bufs=2))
                zbp = actx.enter_context(tc.tile_pool(name="zrb_ps", bufs=1, space="PSUM"))

                avA = avp.tile([128, 512], F32, tag="avA")
                avB = avp.tile([128, 512], F32, tag="avB")
                for (qoff, qsz) in Q_CHUNKS:
                    for i in range(N_KT):
                        koff = i * KT
                        ksz = min(KT, NK - koff)
                        # two 2-head halves so PE streams one half's scores
                        # while ACT exps the other (double-buffered pipeline)
                        halves = []
                        for half in range(2):
                            sc = scp.tile([128, 2, 512], F32, tag=f"sc{half}")
                            for hh in range(2):
                                h = 2 * half + hh
                                nc.tensor.matmul(
                                    sc[0:ksz, hh, 0:qsz],
                                    khT[32 * h:32 * h + 32, koff:koff + ksz],
                                    qhT[32 * h:32 * h + 32, qoff:qoff + qsz],
                                    start=True, stop=True, tile_position=(32 * h, 0))
                            pe = pep.tile([128, 2, 512], F32, tag=f"pe{half}")
                            nc.scalar.activation(pe[0:ksz, :, 0:qsz],
                                                 sc[0:ksz, :, 0:qsz], AF.Exp)
                            halves.append(pe)
                        for h in range(HEADS):
                            av = avA if h < 2 else avB
                            rbase = 64 * (h % 2)
                            nc.tensor.matmul(
                                av[rbase:rbase + 33, 0:qsz],
                                vpack[0:ksz, 132 * i + 33 * h:132 * i + 33 * h + 33],
                                halves[h // 2][0:ksz, h % 2, 0:qsz],
                                start=(i == 0), stop=(i == N_KT - 1),
                                tile_position=(0, rbase),
                                skip_group_check=True)
                    # epilogue: normalize by Z (row 32 / 96 of each bank)
                    for bank, (av, aT) in enumerate(((avA, aTA), (avB, aTB))):
                        zr = zrp.tile([128, 512], F32, tag="zr")
                        nc.vector.reciprocal(zr[32:33, 0:qsz],
                                                         av[32:33, 0:qsz])
                        nc.vector.reciprocal(zr[96:97, 0:qsz],
                                                         av[96:97, 0:qsz])
                        zrb = zbp.tile([128, 512], F32, tag="zrb")
                        nc.tensor.matmul(zrb[0:33, 0:qsz], ones_s[32:33, 0:33],
                                         zr[32:33, 0:qsz], start=True, stop=True,
                                         tile_position=(32, 0))
                        nc.tensor.matmul(zrb[64:97, 0:qsz], ones_s[96:97, 0:33],
                                         zr[96:97, 0:qsz], start=True, stop=True,
                                         tile_position=(96, 64))
                        zrs = zrp.tile([128, 512], F32, tag="zrs")
                        nc.vector.tensor_copy(zrs[0:33, 0:qsz], zrb[0:33, 0:qsz])
                        nc.vector.tensor_copy(zrs[64:97, 0:qsz], zrb[64:97, 0:qsz])
                        nc.vector.tensor_mul(aT[0:33, qoff:qoff + qsz],
                                             av[0:33, 0:qsz], zrs[0:33, 0:qsz])
                        nc.vector.tensor_mul(aT[64:97, qoff:qoff + qsz],
                                             av[64:97, 0:qsz], zrs[64:97, 0:qsz])

            # ---------------- back half ----------------
            with contextlib.ExitStack() as bctx:
                skp = bctx.enter_context(tc.tile_pool(name="skip_pool", bufs=1))
                zp = bctx.enter_context(tc.tile_pool(name="z_ps", bufs=1, space="PSUM"))
                tp = bctx.enter_context(tc.tile_pool(name="t_ps", bufs=1, space="PSUM"))
                hp = bctx.enter_context(tc.tile_pool(name="h_ps", bufs=2, space="PSUM"))
                bwp = bctx.enter_context(tc.tile_pool(name="bk_work", bufs=3))
                bst = bctx.enter_context(tc.tile_pool(name="bk_stats", bufs=1))

                skip_sb = skp.tile([128, TQ], F32)
                nc.sync.dma_start(out=skip_sb[:], in_=xskip[:])

                mv1 = bst.tile([128, 2 * N_QT], F32, tag="mv1")
                mv2 = bst.tile([128, 2 * N_QT], F32, tag="mv2")
                nmu1 = bst.tile([128, N_QT], F32, tag="nmu1")
                rs1 = bst.tile([128, N_QT], F32, tag="rs1")
                nmu2 = bst.tile([128, N_QT], F32, tag="nmu2")
                rs2 = bst.tile([128, N_QT], F32, tag="rs2")

                def chunk_sizes():
                    for j in range(N_QT):
                        off = j * KT
                        yield j, off, min(KT, TQ - off)

                # proj + skip + pre-LN stats; stash z
                for j, off, csz in chunk_sizes():
                    zps = zp.tile([128, 128], F32, tag="zps")
                    nc.tensor.matmul(zps[0:csz, :], aTA[:, off:off + csz], pjA_s[:, :],
                                     start=True, stop=False, skip_group_check=True)
                    nc.tensor.matmul(zps[0:csz, :], aTB[:, off:off + csz], pjB_s[:, :],
                                     start=False, stop=False, skip_group_check=True)
                    if has_proj_b:
                        nc.tensor.matmul(zps[0:csz, :], ones_s[0:1, 0:csz],
                                         pb_s[0:1, :], start=False, stop=False,
                                         skip_group_check=True)
                    nc.tensor.matmul(zps[0:csz, :], skip_sb[:, off:off + csz],
                                     id_s[:, :], is_transpose=True,
                                     start=False, stop=True, skip_group_check=True)
                    bns = bwp.tile([128, 6], F32, tag="bns")
                    nc.vector.bn_stats(bns[0:csz, :], zps[0:csz, :])
                    nc.vector.bn_aggr(mv1[0:csz, 2 * j:2 * j + 2], bns[0:csz, :])
                    nc.vector.tensor_copy(z_sb[0:csz, 128 * j:128 * j + 128], zps[0:csz, :])

                mv1v = mv1[:, :].rearrange("p (t two) -> p t two", two=2)
                nc.vector.tensor_scalar(nmu1[:, :],
                                        mv1v[:, :, 0:1].rearrange("p t o -> p (t o)"),
                                        -1.0, None, OP.mult)
                sd1 = bwp.tile([128, N_QT], F32, tag="sd1")
                nc.scalar.activation(sd1[:, :],
                                     mv1v[:, :, 1:2].rearrange("p t o -> p (t o)"),
                                     AF.Sqrt, bias=eps_s[:, 0:1])
                nc.vector.reciprocal(rs1[:, :], sd1[:, :])

                # MLP per chunk + post-LN stats
                for j, off, csz in chunk_sizes():
                    zln = bwp.tile([128, 128], F32, tag="zln")
                    nc.vector.tensor_scalar(zln[0:csz, :], z_sb[0:csz, 128 * j:128 * j + 128],
                                            nmu1[0:csz, j:j + 1], rs1[0:csz, j:j + 1],
                                            OP.add, OP.mult)
                    trz = tp.tile([128, 128], F32, tag="trz")
                    nc.tensor.matmul(trz[:, 0:csz], zln[0:csz, :], id_s[0:csz, 0:csz],
                                     is_transpose=True, start=True, stop=True)
                    zlnT = bwp.tile([128, 128], F32, tag="zlnT")
                    nc.vector.tensor_copy(zlnT[:, 0:csz], trz[:, 0:csz])
                    hps = hp.tile([128, 2 * D], F32, tag="hps")
                    nc.tensor.matmul(hps[0:csz, :], zlnT[:, 0:csz], w1_s[:, :],
                                     start=True, stop=not has_b1,
                                     skip_group_check=True)
                    if has_b1:
                        nc.tensor.matmul(hps[0:csz, :], ones_s[0:1, 0:csz],
                                         b1_s[0:1, :], start=False, stop=True,
                                         skip_group_check=True)
                    hg = bwp.tile([128, 2 * D], F32, tag="hg")
                    nc.scalar.activation(hg[0:csz, :], hps[0:csz, :], AF.Gelu)
                    mps = zp.tile([128, 128], F32, tag="mps")
                    for bidx, w2s in ((0, w2a_s), (1, w2b_s)):
                        trh = tp.tile([128, 128], F32, tag="trh")
                        nc.tensor.matmul(trh[:, 0:csz],
                                         hg[0:csz, 128 * bidx:128 * bidx + 128],
                                         id_s[0:csz, 0:csz], is_transpose=True,
                                         start=True, stop=True)
                        hgT = bwp.tile([128, 128], F32, tag="hgT")
                        nc.vector.tensor_copy(hgT[:, 0:csz], trh[:, 0:csz])
                        nc.tensor.matmul(mps[0:csz, :], hgT[:, 0:csz], w2s[:, :],
                                         start=(bidx == 0),
                                         stop=(bidx == 1 and not has_b2),
                                         skip_group_check=True)
                    if has_b2:
                        nc.tensor.matmul(mps[0:csz, :], ones_s[0:1, 0:csz],
                                         b2_s[0:1, :], start=False, stop=True,
                                         skip_group_check=True)
                    zr2 = bwp.tile([128, 128], F32, tag="zr2")
                    nc.vector.tensor_add(zr2[0:csz, :], mps[0:csz, :],
                                         z_sb[0:csz, 128 * j:128 * j + 128])
                    nc.vector.tensor_copy(z_sb[0:csz, 128 * j:128 * j + 128], zr2[0:csz, :])
                    bns2 = bwp.tile([128, 6], F32, tag="bns2")
                    nc.vector.bn_stats(bns2[0:csz, :], zr2[0:csz, :])
                    nc.vector.bn_aggr(mv2[0:csz, 2 * j:2 * j + 2], bns2[0:csz, :])

                mv2v = mv2[:, :].rearrange("p (t two) -> p t two", two=2)
                nc.vector.tensor_scalar(nmu2[:, :],
                                        mv2v[:, :, 0:1].rearrange("p t o -> p (t o)"),
                                        -1.0, None, OP.mult)
                sd2 = bwp.tile([128, N_QT], F32, tag="sd2")
                nc.scalar.activation(sd2[:, :],
                                     mv2v[:, :, 1:2].rearrange("p t o -> p (t o)"),
                                     AF.Sqrt, bias=eps_s[:, 0:1])
                nc.vector.reciprocal(rs2[:, :], sd2[:, :])

                for j, off, csz in chunk_sizes():
                    zo = bwp.tile([128, 128], F32, tag="zo")
                    nc.vector.tensor_scalar(zo[0:csz, :], z_sb[0:csz, 128 * j:128 * j + 128],
                                            nmu2[0:csz, j:j + 1], rs2[0:csz, j:j + 1],
                                            OP.add, OP.mult)
                    if has_post:
                        zo2 = bwp.tile([128, 128], F32, tag="zo2")
                        nc.vector.tensor_mul(zo2[0:csz, :], zo[0:csz, :],
                                             pog_s[0:csz, :])
                        nc.vector.tensor_add(zo[0:csz, :], zo2[0:csz, :],
                                             pob_s[0:csz, :])
                    tro = tp.tile([128, 128], F32, tag="tro")
                    nc.tensor.matmul(tro[:, 0:csz], zo[0:csz, :], id_s[0:csz, 0:csz],
                                     is_transpose=True, start=True, stop=True)
                    nc.vector.tensor_copy(outfm[:, off:off + csz], tro[:, 0:csz])

                nc.sync.dma_start(out=y[:], in_=outfm[:])

    return nc


def _host_prep(inputs):
    f = np.float32
    g = {}
    scale = np.float32(DHEAD ** -0.5)
    wq_e = (np.asarray(inputs["ln_q_g"], f)[:, None] * np.asarray(inputs["wq"], f)) * scale
    bq_e = (np.asarray(inputs["ln_q_b"], f) @ np.asarray(inputs["wq"], f)
            + np.asarray(inputs["bq"], f)) * scale
    wk_e = np.asarray(inputs["ln_k_g"], f)[:, None] * np.asarray(inputs["wk"], f)
    bk_e = np.asarray(inputs["ln_k_b"], f) @ np.asarray(inputs["wk"], f) + np.asarray(inputs["bk"], f)
    wv_e = np.asarray(inputs["ln_v_g"], f)[:, None] * np.asarray(inputs["wv"], f)
    bv_e = np.asarray(inputs["ln_v_b"], f) @ np.asarray(inputs["wv"], f) + np.asarray(inputs["bv"], f)

    proj_w = np.asarray(inputs["proj_w"], f)
    pjA = np.zeros((128, 128), f)
    pjB = np.zeros((128, 128), f)
    pjA[0:32] = proj_w[0:32]
    pjA[64:96] = proj_w[32:64]
    pjB[0:32] = proj_w[64:96]
    pjB[64:96] = proj_w[96:128]

    pre_g = np.asarray(inputs["pre_g"], f)
    pre_b = np.asarray(inputs["pre_b"], f)
    w1_e = pre_g[:, None] * np.asarray(inputs["mlp_w1"], f)
    b1_e = pre_b @ np.asarray(inputs["mlp_w1"], f) + np.asarray(inputs["mlp_b1"], f)
    w2 = np.asarray(inputs["mlp_w2"], f)
    b2_e = np.asarray(inputs["mlp_b2"], f)
    proj_b = np.asarray(inputs["proj_b"], f)
    post_g = np.asarray(inputs["post_g"], f)
    post_b = np.asarray(inputs["post_b"], f)

    def bcast(v, n=128):
        return np.ascontiguousarray(np.broadcast_to(v[None, :], (128, n)), f)

    g["wq"], g["wk"], g["wv"] = map(np.ascontiguousarray, (wq_e, wk_e, wv_e))
    g["csq"] = bcast(wq_e.sum(0))
    g["csk"] = bcast(wk_e.sum(0))
    g["csv"] = bcast(wv_e.sum(0))
    g["bqb"] = bcast(bq_e)
    g["bkb"] = bcast(bk_e)
    g["bvb"] = bcast(bv_e)
    g["identity"] = np.eye(128, dtype=f)
    g["onesc"] = np.ones((128, 128), f)
    g["pjA"], g["pjB"] = pjA, pjB
    g["w1"] = np.ascontiguousarray(w1_e)
    g["w2a"] = np.ascontiguousarray(w2[0:128])
    g["w2b"] = np.ascontiguousarray(w2[128:256])
    g["pbrow"] = np.ascontiguousarray(proj_b[None, :])
    g["b1row"] = np.ascontiguousarray(b1_e[None, :])
    g["b2row"] = np.ascontiguousarray(b2_e[None, :])

    flags = {
        "has_proj_b": bool(np.any(proj_b != 0)),
        "has_b1": bool(np.any(b1_e != 0)),
        "has_b2": bool(np.any(b2_e != 0)),
        "has_post": not (np.allclose(post_g, 1.0) and np.allclose(post_b, 0.0)),
    }
    if flags["has_post"]:
        g["pogb"] = bcast(post_g)
        g["pobb"] = bcast(post_b)
    return g, flags


_CACHE = {}


def kernel(**inputs):
    f = np.float32
    q = np.asarray(inputs["q"], f)
    k = np.asarray(inputs["k"], f)
    v = np.asarray(inputs["v"], f)
    skip = np.asarray(inputs["skip"], f)

    consts, flags = _host_prep(inputs)

    starts = [0, 938, 1876, 2813]
    lens = [938, 938, 937, 937]

    in_maps = []
    for c in range(8):
        b, s = c // 4, c % 4
        qfm = np.ascontiguousarray(q[b].transpose(1, 0, 2).reshape(128, NQ_FULL))
        sfm = np.ascontiguousarray(skip[b].transpose(1, 0, 2).reshape(128, NQ_FULL))
        kfm = np.ascontiguousarray(k[b].transpose(1, 0, 2, 3).reshape(128, NK))
        vfm = np.ascontiguousarray(v[b].transpose(1, 0, 2, 3).reshape(128, NK))
        xq = np.zeros((128, TQ), f)
        xs = np.zeros((128, TQ), f)
        xq[:, :lens[s]] = qfm[:, starts[s]:starts[s] + lens[s]]
        xs[:, :lens[s]] = sfm[:, starts[s]:starts[s] + lens[s]]
        m = {"xq": xq, "xk": kfm, "xv": vfm, "xskip": xs}
        m.update(consts)
        in_maps.append(m)

    key = tuple(sorted(flags.items()))
    if key not in _CACHE:
        _CACHE[key] = build_program(flags)
    nc = _CACHE[key]

    _install_compile_patch()
    res = bass_utils.run_bass_kernel_spmd(nc, in_maps, core_ids=list(range(8)))

    full = np.zeros((B, 128, NQ_FULL), f)
    for c in range(8):
        b, s = c // 4, c % 4
        full[b][:, starts[s]:starts[s] + lens[s]] = res.results[c]["y"][:, :lens[s]]
    return np.ascontiguousarray(
        full.reshape(B, 128, N, M).transpose(0, 2, 1, 3))



# revision 6
# speedup vs baseline: 1.6107x; 1.6107x over previous
"""Trainium2 Bass kernel for nn_CrossAttention (B=2,N=6,D=128,M=625,H=28,W=60, 4 heads x 32).

Sharding: 8 cores = 2 batches x 4 query-token shards. Zero collectives.

Key optimizations over the f32 baseline:
- all matmul operands bf16 (fp32 streams cost 4 cyc/row vs 1 on the PE)
- LN mean-centering folded into the projection weights on the host:
  (x - mu) @ W == x @ (W - colsum(W)/128), so K/V need no LN-apply pass at all
- K's LN scale alpha_s folded into the exp (per-partition activation scale);
  softmax-invariant per-query score terms (K-side biases) dropped entirely
- V's LN scale folded into the vpack copy; V bias folded into proj_b (host)
- exp split across engines: heads 0,1 exact exp on ScalarE, heads 2,3 via a
  DVE linear-interp exp2 bit trick (int16 bits viewed as bf16), halving the
  activation bottleneck
- per-token LN stats via PE transpose + DVE bn_stats (no 1-col stats matmuls)
"""

import numpy as np
import ml_dtypes

import concourse.bass as bass
import concourse.mybir as mybir
import concourse.tile as tile
from concourse import bass_utils
from concourse.vector_clock import ScopedClock, VectorClock
from concourse.tile_scheduler import N_PROCS

F32 = mybir.dt.float32
BF16 = mybir.dt.bfloat16
I16 = mybir.dt.int16
AF = mybir.ActivationFunctionType
OP = mybir.AluOpType

B, N, D, M, H, W = 2, 6, 128, 625, 28, 60
HEADS, DHEAD = 4, 32
NQ_FULL = N * M            # 3750
NK = N * H * W             # 10080
TQ = 938                   # padded per-core query shard
EPS = 1e-5
EXP_C = 184.66496          # 128 * log2(e): bf16-bits exp trick multiplier
EXP_B = 16256.0            # 127 * 128: bf16 exponent bias in bit space

KT = 128
N_KT = (NK + KT - 1) // KT          # 79 (last = 96)
N_QT = (TQ + KT - 1) // KT          # 8  (last = 42)
Q_CHUNKS = [(0, 512), (512, TQ - 512)]
KC = 512
K_CHUNKS = [(o, min(KC, NK - o)) for o in range(0, NK, KC)]   # 20 (last = 352)


def _k_tiles():
    for j in range(N_KT):
        off = j * KT
        yield j, off, min(KT, NK - off)


def _q_tiles():
    for j in range(N_QT):
        off = j * KT
        yield j, off, min(KT, TQ - off)


def _split_multiwait_json(bir_json: bytes) -> bytes:
    """This walrus build allows only one sync-wait per instruction: move
    extra on_wait entries onto EventSemaphore instructions inserted just
    before the owner (same engine, so ordering is preserved)."""
    import json
    bir = json.loads(bir_json)
    for fn in bir["functions"]:
        for blk in fn["blocks"]:
            out = []
            for ins in blk["instructions"]:
                si = ins.get("sync_info")
                waits = (si or {}).get("on_wait") or []
                if len(waits) > 1:
                    for wi, w in enumerate(waits[:-1]):
                        out.append({
                            "debug": ins.get("debug", 0),
                            "engine": ins["engine"],
                            "ins": [], "outs": [],
                            "name": f"{ins['name']}-xw{wi}",
                            "opcode": "EventSemaphore",
                            "sync_info": {"on_update": [], "on_wait": [w]},
                        })
                    si["on_wait"] = [waits[-1]]
                out.append(ins)
            blk["instructions"] = out
    return json.dumps(bir).encode()


def _install_compile_patch():
    from concourse import bass_utils as bu
    if getattr(bu, "_mw_patched", False):
        return
    orig = bu.compile_bir_kernel

    def patched(bir_json, tmpdir, neff_name="file.neff"):
        return orig(_split_multiwait_json(bir_json), tmpdir, neff_name)

    bu.compile_bir_kernel = patched
    bu._mw_patched = True
    try:
        from concourse import bass2jax
        if getattr(bass2jax, "compile_bir_kernel", None) is orig:
            bass2jax.compile_bir_kernel = patched
    except ImportError:
        pass


class _SplitDrainTileContext(tile.TileContext):
    """This walrus build rejects >1 sem wait on a Drain; split the exit
    drain's waits across per-proc drains (one wait each)."""

    def _drain_and_barrier(self, tick_clock, wait_clock):
        full = tick_clock.global_clock
        for p in range(N_PROCS):
            mask = VectorClock([(1 << 30) if i == p else 0 for i in range(N_PROCS)])
            partial = full.copy()
            partial.elementwise_min(mask)
            d = self.nc.sync.drain()
            wait_clock.add_sem_waits(d.ins, ScopedClock({None: partial}))
        self.nc.all_engine_barrier()
        assert self.sems is not None
        popped = self.nc._tile_sem_poison_stack.pop()
        assert popped is self._sem_poison
        self.nc.clear_and_free_semaphores(list(self.sems.allocated().values()))
        self.nc.all_engine_barrier()


def _ln_alpha(nc, wp, mv, n_tiles, al, eps_ap, al184=None, nmu=None):
    """From interleaved bn_aggr stats mv [128, 2*n]: al = rsqrt(var+eps),
    optionally al184 = al*EXP_C and nmu = -mean."""
    mvv = mv[:, :].rearrange("p (t two) -> p t two", two=2)
    var_ap = mvv[:, :, 1:2].rearrange("p t o -> p (t o)")
    sd = wp.tile([128, n_tiles], F32, tag="lnsd")
    nc.scalar.activation(sd[:, :], var_ap, AF.Sqrt, bias=eps_ap)
    nc.vector.reciprocal(al[:, :], sd[:, :])
    if al184 is not None:
        nc.vector.tensor_scalar(al184[:, :], al[:, :], EXP_C, None, OP.mult)
    if nmu is not None:
        nc.vector.tensor_scalar(
            nmu[:, :], mvv[:, :, 0:1].rearrange("p t o -> p (t o)"),
            -1.0, None, OP.mult)


def build_program(host):
    nc = bass.Bass()

    def inp(name, shape, dt=BF16):
        return nc.dram_tensor(name, list(shape), dt, kind="ExternalInput")

    xq = inp("xq", (128, TQ))
    xk = inp("xk", (128, NK))
    xv = inp("xv", (128, NK))
    xskip = inp("xskip", (128, N_QT * 128), F32)
    wqc = inp("wqc", (128, 128))
    wkc = inp("wkc", (128, 128))
    wvc = inp("wvc", (128, 128))
    pjA = inp("pjA", (128, 128))
    pjB = inp("pjB", (128, 128))
    w1a = inp("w1a", (128, 128))
    w1b = inp("w1b", (128, 128))
    w2a = inp("w2a", (128, 128))
    w2b = inp("w2b", (128, 128))
    id16 = inp("id16", (128, 128))
    id32 = inp("id32", (128, 128), F32)
    ones16 = inp("ones16", (128, 128))
    y = nc.dram_tensor("y", [128, TQ], F32, kind="ExternalOutput")

    has_bq = host["has_bq"]
    has_b1 = host["has_b1"]
    has_b2 = host["has_b2"]
    has_post = host["has_post"]
    if has_bq:
        bqcol = inp("bqcol", (128, 1), F32)
    if has_b1:
        b1acol = inp("b1acol", (128, 1), F32)
        b1bcol = inp("b1bcol", (128, 1), F32)
    if has_b2:
        b2row = inp("b2row", (1, 128))
    if has_post:
        pogb = inp("pogb", (128, 128), F32)
        pobb = inp("pobb", (128, 128), F32)

    with _SplitDrainTileContext(nc) as tc:
        import contextlib
        with contextlib.ExitStack() as ctx:
            cpool = ctx.enter_context(tc.tile_pool(name="consts", bufs=1))
            big = ctx.enter_context(tc.tile_pool(name="big", bufs=1))

            def load_const(t, shape, dt=BF16):
                s = cpool.tile(list(shape), dt, tag=t.name)
                nc.sync.dma_start(out=s[:], in_=t[:])
                return s

            wqc_s = load_const(wqc, (128, 128))
            wkc_s = load_const(wkc, (128, 128))
            wvc_s = load_const(wvc, (128, 128))
            pjA_s = load_const(pjA, (128, 128))
            pjB_s = load_const(pjB, (128, 128))
            w1a_s = load_const(w1a, (128, 128))
            w1b_s = load_const(w1b, (128, 128))
            w2a_s = load_const(w2a, (128, 128))
            w2b_s = load_const(w2b, (128, 128))
            id16_s = load_const(id16, (128, 128))
            id32_s = load_const(id32, (128, 128), F32)
            ones16_s = load_const(ones16, (128, 128))
            bq_s = load_const(bqcol, (128, 1), F32) if has_bq else None
            b1a_s = load_const(b1acol, (128, 1), F32) if has_b1 else None
            b1b_s = load_const(b1bcol, (128, 1), F32) if has_b1 else None
            b2_s = load_const(b2row, (1, 128)) if has_b2 else None
            if has_post:
                pog_s = load_const(pogb, (128, 128), F32)
                pob_s = load_const(pobb, (128, 128), F32)

            eps_s = cpool.tile([128, 1], F32, tag="eps")
            nc.vector.memset(eps_s[:, :], EPS)

            xq_sb = big.tile([128, TQ], BF16, tag="xq_sb")
            xk_sb = big.tile([128, NK], BF16, tag="xk_sb")
            xv_sb = big.tile([128, NK], BF16, tag="xv_sb")
            skip_sb = big.tile([128, N_QT * 128], F32, tag="skip_sb")
            khT = big.tile([128, NK], BF16, tag="khT")
            qhT = big.tile([128, TQ], BF16, tag="qhT")
            vpack = big.tile([128, N_KT * 132], BF16, tag="vpack")
            aTA = big.tile([128, TQ], BF16, tag="aTA")
            aTB = big.tile([128, TQ], BF16, tag="aTB")
            z_sb = big.tile([128, N_QT * 128], F32, tag="z_sb")
            outfm = big.tile([128, TQ], F32, tag="outfm")
            qn_sb = big.tile([128, N_QT * 128], BF16, tag="qn_sb")
            qn_fm = big.tile([128, TQ], BF16, tag="qn_fm")
            alK = big.tile([128, N_KT], F32, tag="alK")
            al184K = big.tile([128, N_KT], F32, tag="al184K")
            alV = big.tile([128, N_KT], F32, tag="alV")
            alQ = big.tile([128, N_QT], F32, tag="alQ")
            mvK = big.tile([128, 2 * N_KT], F32, tag="mvK")
            mvV = big.tile([128, 2 * N_KT], F32, tag="mvV")
            mvQ = big.tile([128, 2 * N_QT], F32, tag="mvQ")

            nc.sync.dma_start(out=xq_sb[:], in_=xq[:])
            nc.sync.dma_start(out=xk_sb[:], in_=xk[:])
            nc.sync.dma_start(out=xv_sb[:], in_=xv[:])
            nc.sync.dma_start(out=skip_sb[:], in_=xskip[:])

            # vpack Z columns (softmax denominator trick); junk rows of aT
            nc.vector.memset(
                vpack[:, :].rearrange("p (t h c) -> p t h c", h=HEADS, c=33)[:, :, :, 32:33],
                1.0)
            nc.vector.memset(aTA[:, :], 0.0)
            nc.vector.memset(aTB[:, :], 0.0)

            # ---------------- Q: full LN (token-major) + projection ----------
            with contextlib.ExitStack() as qctx:
                qps = qctx.enter_context(tc.tile_pool(name="q_ps", bufs=1, space="PSUM"))
                qtr = qctx.enter_context(tc.tile_pool(name="q_tr", bufs=2, space="PSUM"))
                qpj = qctx.enter_context(tc.tile_pool(name="q_pj", bufs=2, space="PSUM"))
                qwp = qctx.enter_context(tc.tile_pool(name="q_wp", bufs=3))

                qT = qps.tile([128, N_QT, 128], BF16, tag="qT")
                for j, off, tsz in _q_tiles():
                    nc.tensor.matmul(qT[0:tsz, j, :], xq_sb[:, off:off + tsz],
                                     id16_s[:, :], is_transpose=True,
                                     start=True, stop=True)
                for j, off, tsz in _q_tiles():
                    b6 = qwp.tile([128, 6], F32, tag="qb6")
                    nc.vector.bn_stats(b6[0:tsz, :], qT[0:tsz, j, :])
                    nc.vector.bn_aggr(mvQ[0:tsz, 2 * j:2 * j + 2], b6[0:tsz, :])
                _ln_alpha(nc, qwp, mvQ, N_QT, alQ, eps_s[:, 0:1])
                for j, off, tsz in _q_tiles():
                    nc.vector.tensor_scalar(qn_sb[0:tsz, 128 * j:128 * j + 128],
                                            qT[0:tsz, j, :], alQ[0:tsz, j:j + 1],
                                            None, OP.mult)
                for j, off, tsz in _q_tiles():
                    qb = qtr.tile([128, 128], BF16, tag="qb")
                    nc.tensor.matmul(qb[:, 0:tsz], qn_sb[0:tsz, 128 * j:128 * j + 128],
                                     id16_s[0:tsz, 0:tsz], is_transpose=True,
                                     start=True, stop=True)
                    nc.scalar.copy(qn_fm[:, off:off + tsz], qb[:, 0:tsz])
                for qoff, qsz in Q_CHUNKS:
                    qh = qpj.tile([128, 512], F32, tag="qh")
                    nc.tensor.matmul(qh[0:128, 0:qsz], wqc_s[:, :],
                                     qn_fm[:, qoff:qoff + qsz], start=True, stop=True)
                    if has_bq:
                        nc.scalar.activation(qhT[:, qoff:qoff + qsz], qh[0:128, 0:qsz],
                                             AF.Identity, bias=bq_s[:, 0:1])
                    else:
                        nc.scalar.copy(qhT[:, qoff:qoff + qsz], qh[0:128, 0:qsz])

            # ---------------- K: centered projection + LN scale columns ------
            with contextlib.ExitStack() as kctx:
                kpj = kctx.enter_context(tc.tile_pool(name="k_pj", bufs=2, space="PSUM"))
                ktr = kctx.enter_context(tc.tile_pool(name="k_tr", bufs=2, space="PSUM"))
                kwp = kctx.enter_context(tc.tile_pool(name="k_wp", bufs=3))

                for coff, csz in K_CHUNKS:
                    pp = kpj.tile([128, KC], F32, tag="pp")
                    nc.tensor.matmul(pp[0:128, 0:csz], wkc_s[:, :],
                                     xk_sb[:, coff:coff + csz], start=True, stop=True)
                    nc.scalar.copy(khT[:, coff:coff + csz], pp[0:128, 0:csz])

                for g in range(0, N_KT, 4):
                    js = [(j, j * KT, min(KT, NK - j * KT))
                          for j in range(g, min(g + 4, N_KT))]
                    tp = ktr.tile([128, 4, 128], BF16, tag="tp")
                    for t, (j, off, tsz) in enumerate(js):
                        nc.tensor.matmul(tp[0:tsz, t, :], xk_sb[:, off:off + tsz],
                                         id16_s[:, :], is_transpose=True,
                                         start=True, stop=True)
                    for t, (j, off, tsz) in enumerate(js):
                        b6 = kwp.tile([128, 6], F32, tag="kb6")
                        nc.vector.bn_stats(b6[0:tsz, :], tp[0:tsz, t, :])
                        nc.vector.bn_aggr(mvK[0:tsz, 2 * j:2 * j + 2],
                                          b6[0:tsz, :])
                _ln_alpha(nc, kwp, mvK, N_KT, alK, eps_s[:, 0:1], al184=al184K)

            # ---------------- V: centered projection, alpha into vpack -------
            with contextlib.ExitStack() as vctx:
                vtr = vctx.enter_context(tc.tile_pool(name="v_tr", bufs=2, space="PSUM"))
                vpj = vctx.enter_context(tc.tile_pool(name="v_pj", bufs=4, space="PSUM"))
                vwp = vctx.enter_context(tc.tile_pool(name="v_wp", bufs=3))

                for g in range(0, N_KT, 4):
                    js = [(j, j * KT, min(KT, NK - j * KT))
                          for j in range(g, min(g + 4, N_KT))]
                    tp = vtr.tile([128, 4, 128], BF16, tag="vtp")
                    for t, (j, off, tsz) in enumerate(js):
                        nc.tensor.matmul(tp[0:tsz, t, :], xv_sb[:, off:off + tsz],
                                         id16_s[:, :], is_transpose=True,
                                         start=True, stop=True)
                    for t, (j, off, tsz) in enumerate(js):
                        b6 = vwp.tile([128, 6], F32, tag="vb6")
                        nc.vector.bn_stats(b6[0:tsz, :], tp[0:tsz, t, :])
                        nc.vector.bn_aggr(mvV[0:tsz, 2 * j:2 * j + 2],
                                          b6[0:tsz, :])
                _ln_alpha(nc, vwp, mvV, N_KT, alV, eps_s[:, 0:1])

                for j, off, tsz in _k_tiles():
                    vp = vpj.tile([128, 128], F32, tag="vp")
                    nc.tensor.matmul(vp[0:tsz, :], xv_sb[:, off:off + tsz],
                                     wvc_s[:, :], start=True, stop=True)
                    dv = vpack[:, 132 * j:132 * j + 132].rearrange(
                        "p (h c) -> p h c", h=HEADS)[:, :, 0:DHEAD]
                    nc.scalar.activation(
                        dv[0:tsz, :, :],
                        vp[0:tsz, :].rearrange("p (h c) -> p h c", c=DHEAD),
                        AF.Copy, scale=alV[0:tsz, j:j + 1])

            # ---------------- attention ----------------
            with contextlib.ExitStack() as actx:
                scp = actx.enter_context(tc.tile_pool(name="sc_ps", bufs=1, space="PSUM"))
                avp = actx.enter_context(tc.tile_pool(name="av_ps", bufs=1, space="PSUM"))
                zbp = actx.enter_context(tc.tile_pool(name="zrb_ps", bufs=1, space="PSUM"))
                pep = actx.enter_context(tc.tile_pool(name="pexp", bufs=2))
                zrp = actx.enter_context(tc.tile_pool(name="zr", bufs=2))

                avA = avp.tile([128, 512], F32, tag="avA")
                avB = avp.tile([128, 512], F32, tag="avB")
                for (qoff, qsz) in Q_CHUNKS:
                    for j, koff, ksz in _k_tiles():
                        pes = []
                        for half in range(2):
                            sc = scp.tile([128, 2, 512], F32, tag=f"sc{half}")
                            for hh in range(2):
                                h = 2 * half + hh
                                nc.tensor.matmul(
                                    sc[0:ksz, hh, 0:qsz],
                                    khT[32 * h:32 * h + 32, koff:koff + ksz],
                                    qhT[32 * h:32 * h + 32, qoff:qoff + qsz],
                                    start=True, stop=True, tile_position=(32 * h, 0))
                            pe = pep.tile([128, 2, 512], BF16, tag=f"pe{half}")
                            if half == 0:
                                nc.scalar.activation(pe[0:ksz, :, 0:qsz],
                                                     sc[0:ksz, :, 0:qsz], AF.Exp,
                                                     scale=alK[0:ksz, j:j + 1])
                            else:
                                nc.vector.tensor_scalar(
                                    pe[0:ksz, :, 0:qsz].bitcast(I16),
                                    sc[0:ksz, :, 0:qsz],
                                    al184K[0:ksz, j:j + 1], EXP_B,
                                    OP.mult, OP.add)
                            pes.append(pe)
                        for h in range(HEADS):
                            av = avA if h < 2 else avB
                            rbase = 64 * (h % 2)
                            nc.tensor.matmul(
                                av[rbase:rbase + 33, 0:qsz],
                                vpack[0:ksz, 132 * j + 33 * h:132 * j + 33 * h + 33],
                                pes[h // 2][0:ksz, h % 2, 0:qsz],
                                start=(j == 0), stop=(j == N_KT - 1),
                                tile_position=(0, rbase),
                                skip_group_check=True)
                    # epilogue: normalize by Z (row 32 / 96 of each bank)
                    for av, aT in ((avA, aTA), (avB, aTB)):
                        zr = zrp.tile([128, 512], BF16, tag="zr")
                        with nc.allow_low_precision(reason="1/Z softmax scale; bf16 ok"):
                            nc.vector.reciprocal(zr[32:33, 0:qsz], av[32:33, 0:qsz])
                            nc.vector.reciprocal(zr[96:97, 0:qsz], av[96:97, 0:qsz])
                        zrb = zbp.tile([128, 512], F32, tag="zrb")
                        nc.tensor.matmul(zrb[0:33, 0:qsz], ones16_s[32:33, 0:33],
                                         zr[32:33, 0:qsz], start=True, stop=True,
                                         tile_position=(32, 0))
                        nc.tensor.matmul(zrb[64:97, 0:qsz], ones16_s[96:97, 0:33],
                                         zr[96:97, 0:qsz], start=True, stop=True,
                                         tile_position=(96, 64))
                        zrs = zrp.tile([128, 512], BF16, tag="zrs")
                        nc.vector.tensor_copy(zrs[0:33, 0:qsz], zrb[0:33, 0:qsz])
                        nc.vector.tensor_copy(zrs[64:97, 0:qsz], zrb[64:97, 0:qsz])
                        nc.vector.tensor_mul(aT[0:33, qoff:qoff + qsz],
                                             av[0:33, 0:qsz], zrs[0:33, 0:qsz])
                        nc.vector.tensor_mul(aT[64:97, qoff:qoff + qsz],
                                             av[64:97, 0:qsz], zrs[64:97, 0:qsz])

            # ---------------- back half ----------------
            with contextlib.ExitStack() as bctx:
                zp = bctx.enter_context(tc.tile_pool(name="z_ps", bufs=1, space="PSUM"))
                tp = bctx.enter_context(tc.tile_pool(name="t_ps", bufs=1, space="PSUM"))
                hp = bctx.enter_context(tc.tile_pool(name="h_ps", bufs=1, space="PSUM"))
                bwp = bctx.enter_context(tc.tile_pool(name="bk_work", bufs=3))
                bst = bctx.enter_context(tc.tile_pool(name="bk_stats", bufs=1))

                mv1 = bst.tile([128, 2 * N_QT], F32, tag="mv1")
                mv2 = bst.tile([128, 2 * N_QT], F32, tag="mv2")
                nmu1 = bst.tile([128, N_QT], F32, tag="nmu1")
                rs1 = bst.tile([128, N_QT], F32, tag="rs1")
                nmu2 = bst.tile([128, N_QT], F32, tag="nmu2")
                rs2 = bst.tile([128, N_QT], F32, tag="rs2")

                # proj + skip + pre-LN stats
                for j, off, csz in _q_tiles():
                    zps = zp.tile([128, 128], F32, tag="zps")
                    nc.tensor.matmul(zps[0:csz, :], aTA[:, off:off + csz], pjA_s[:, :],
                                     start=True, stop=False, skip_group_check=True)
                    nc.tensor.matmul(zps[0:csz, :], aTB[:, off:off + csz], pjB_s[:, :],
                                     start=False, stop=True, skip_group_check=True)
                    nc.vector.tensor_add(z_sb[0:csz, 128 * j:128 * j + 128],
                                         zps[0:csz, :],
                                         skip_sb[0:csz, 128 * j:128 * j + 128])
                    bns = bwp.tile([128, 6], F32, tag="bns")
                    nc.vector.bn_stats(bns[0:csz, :], z_sb[0:csz, 128 * j:128 * j + 128])
                    nc.vector.bn_aggr(mv1[0:csz, 2 * j:2 * j + 2], bns[0:csz, :])
                _ln_alpha(nc, bwp, mv1, N_QT, rs1, eps_s[:, 0:1], nmu=nmu1)

                # MLP per chunk + post-LN stats
                for j, off, csz in _q_tiles():
                    zln = bwp.tile([128, 128], BF16, tag="zln")
                    nc.vector.tensor_scalar(zln[0:csz, :], z_sb[0:csz, 128 * j:128 * j + 128],
                                            nmu1[0:csz, j:j + 1], rs1[0:csz, j:j + 1],
                                            OP.add, OP.mult)
                    trz = tp.tile([128, 128], BF16, tag="trz")
                    nc.tensor.matmul(trz[:, 0:csz], zln[0:csz, :], id16_s[0:csz, 0:csz],
                                     is_transpose=True, start=True, stop=True)
                    zlnT = bwp.tile([128, 128], BF16, tag="zlnT")
                    nc.vector.tensor_copy(zlnT[:, 0:csz], trz[:, 0:csz])
                    hg = bwp.tile([128, 2, 128], BF16, tag="hg")
                    for bi, w1s in ((0, w1a_s), (1, w1b_s)):
                        hps = hp.tile([128, 128], F32, tag=f"hps{bi}")
                        nc.tensor.matmul(hps[0:128, 0:csz], w1s[:, :], zlnT[:, 0:csz],
                                         start=True, stop=True)
                        gb = (b1a_s if bi == 0 else b1b_s)
                        nc.scalar.activation(hg[:, bi, 0:csz], hps[0:128, 0:csz],
                                             AF.Gelu,
                                             bias=(gb[:, 0:1] if has_b1 else 0.0))
                    mps = zp.tile([128, 128], F32, tag="mps")
                    nc.tensor.matmul(mps[0:csz, :], hg[:, 0, 0:csz], w2a_s[:, :],
                                     start=True, stop=False, skip_group_check=True)
                    nc.tensor.matmul(mps[0:csz, :], hg[:, 1, 0:csz], w2b_s[:, :],
                                     start=False, stop=not has_b2,
                                     skip_group_check=True)
                    if has_b2:
                        nc.tensor.matmul(mps[0:csz, :], ones16_s[0:1, 0:csz],
                                         b2_s[0:1, :], start=False, stop=True,
                                         skip_group_check=True)
                    zr2 = bwp.tile([128, 128], F32, tag="zr2")
                    nc.vector.tensor_add(zr2[0:csz, :], mps[0:csz, :],
                                         z_sb[0:csz, 128 * j:128 * j + 128])
                    nc.vector.tensor_copy(z_sb[0:csz, 128 * j:128 * j + 128], zr2[0:csz, :])
                    bns2 = bwp.tile([128, 6], F32, tag="bns2")
                    nc.vector.bn_stats(bns2[0:csz, :], zr2[0:csz, :])
                    nc.vector.bn_aggr(mv2[0:csz, 2 * j:2 * j + 2], bns2[0:csz, :])
                _ln_alpha(nc, bwp, mv2, N_QT, rs2, eps_s[:, 0:1], nmu=nmu2)

                for j, off, csz in _q_tiles():
                    zo = bwp.tile([128, 128], F32, tag="zo")
                    nc.vector.tensor_scalar(zo[0:csz, :], z_sb[0:csz, 128 * j:128 * j + 128],
                                            nmu2[0:csz, j:j + 1], rs2[0:csz, j:j + 1],
                                            OP.add, OP.mult)
                    if has_post:
                        zo2 = bwp.tile([128, 128], F32, tag="zo2")
                        nc.vector.tensor_mul(zo2[0:csz, :], zo[0:csz, :],
                                             pog_s[0:csz, :])
                        nc.vector.tensor_add(zo[0:csz, :], zo2[0:csz, :],
                                             pob_s[0:csz, :])
                    tro = tp.tile([128, 128], F32, tag="tro")
                    nc.tensor.matmul(tro[:, 0:csz], zo[0:csz, :], id32_s[0:csz, 0:csz],
                                     is_transpose=True, start=True, stop=True)
                    nc.vector.tensor_copy(outfm[:, off:off + csz], tro[:, 0:csz])

                nc.sync.dma_start(out=y[:], in_=outfm[:])

    return nc


def _host_prep(inputs):
    f = np.float32
    bf = ml_dtypes.bfloat16
    g = {}
    scale = np.float32(DHEAD ** -0.5)
    wq_e = (np.asarray(inputs["ln_q_g"], f)[:, None] * np.asarray(inputs["wq"], f)) * scale
    bq_e = (np.asarray(inputs["ln_q_b"], f) @ np.asarray(inputs["wq"], f)
            + np.asarray(inputs["bq"], f)) * scale
    wk_e = np.asarray(inputs["ln_k_g"], f)[:, None] * np.asarray(inputs["wk"], f)
    wv_e = np.asarray(inputs["ln_v_g"], f)[:, None] * np.asarray(inputs["wv"], f)
    bv_e = (np.asarray(inputs["ln_v_b"], f) @ np.asarray(inputs["wv"], f)
            + np.asarray(inputs["bv"], f))
    # mean-centering folded into weights: (x - mu) @ W == x @ (W - colsum/D)
    wqc = wq_e - wq_e.sum(0, keepdims=True) / D
    wkc = wk_e - wk_e.sum(0, keepdims=True) / D
    wvc = wv_e - wv_e.sum(0, keepdims=True) / D

    proj_w = np.asarray(inputs["proj_w"], f)
    proj_b_eff = np.asarray(inputs["proj_b"], f) + bv_e @ proj_w
    pjA = np.zeros((128, 128), f)
    pjB = np.zeros((128, 128), f)
    pjA[0:32] = proj_w[0:32]
    pjA[64:96] = proj_w[32:64]
    pjB[0:32] = proj_w[64:96]
    pjB[64:96] = proj_w[96:128]

    pre_g = np.asarray(inputs["pre_g"], f)
    pre_b = np.asarray(inputs["pre_b"], f)
    w1_e = pre_g[:, None] * np.asarray(inputs["mlp_w1"], f)
    b1_e = pre_b @ np.asarray(inputs["mlp_w1"], f) + np.asarray(inputs["mlp_b1"], f)
    w2 = np.asarray(inputs["mlp_w2"], f)
    b2_e = np.asarray(inputs["mlp_b2"], f)
    post_g = np.asarray(inputs["post_g"], f)
    post_b = np.asarray(inputs["post_b"], f)

    g["wqc"] = np.ascontiguousarray(wqc.astype(bf))
    g["wkc"] = np.ascontiguousarray(wkc.astype(bf))
    g["wvc"] = np.ascontiguousarray(wvc.astype(bf))
    g["pjA"] = pjA.astype(bf)
    g["pjB"] = pjB.astype(bf)
    g["w1a"] = np.ascontiguousarray(w1_e[:, 0:128].astype(bf))
    g["w1b"] = np.ascontiguousarray(w1_e[:, 128:256].astype(bf))
    g["w2a"] = np.ascontiguousarray(w2[0:128].astype(bf))
    g["w2b"] = np.ascontiguousarray(w2[128:256].astype(bf))
    g["id16"] = np.eye(128, dtype=bf)
    g["id32"] = np.eye(128, dtype=f)
    g["ones16"] = np.ones((128, 128), bf)

    flags = {
        "has_bq": bool(np.any(bq_e != 0)),
        "has_b1": bool(np.any(b1_e != 0)),
        "has_b2": bool(np.any(b2_e != 0)),
        "has_post": not (np.allclose(post_g, 1.0) and np.allclose(post_b, 0.0)),
    }
    if flags["has_bq"]:
        g["bqcol"] = np.ascontiguousarray(bq_e[:, None], dtype=f)
    if flags["has_b1"]:
        g["b1acol"] = np.ascontiguousarray(b1_e[0:128, None], dtype=f)
        g["b1bcol"] = np.ascontiguousarray(b1_e[128:256, None], dtype=f)
    if flags["has_b2"]:
        g["b2row"] = np.ascontiguousarray(b2_e[None, :].astype(bf))
    if flags["has_post"]:
        g["pogb"] = np.ascontiguousarray(np.broadcast_to(post_g[None, :], (128, 128)), f)
        g["pobb"] = np.ascontiguousarray(np.broadcast_to(post_b[None, :], (128, 128)), f)
    return g, flags, proj_b_eff


STARTS = [0, 938, 1876, 2813]
LENS = [938, 938, 937, 937]


def _make_in_maps(inputs):
    f = np.float32
    bf = ml_dtypes.bfloat16
    q = np.asarray(inputs["q"], f)
    k = np.asarray(inputs["k"], f)
    v = np.asarray(inputs["v"], f)
    skip = np.asarray(inputs["skip"], f)
    consts, flags, proj_b_eff = _host_prep(inputs)

    in_maps = []
    for c in range(8):
        b, s = c // 4, c % 4
        qfm = np.ascontiguousarray(q[b].transpose(1, 0, 2).reshape(128, NQ_FULL))
        sfm = np.ascontiguousarray(skip[b].transpose(1, 0, 2).reshape(128, NQ_FULL))
        kfm = np.ascontiguousarray(k[b].transpose(1, 0, 2, 3).reshape(128, NK))
        vfm = np.ascontiguousarray(v[b].transpose(1, 0, 2, 3).reshape(128, NK))
        xq = np.zeros((128, TQ), bf)
        xq[:, :LENS[s]] = qfm[:, STARTS[s]:STARTS[s] + LENS[s]].astype(bf)
        sk = np.zeros((128, TQ), f)
        sk[:, :LENS[s]] = sfm[:, STARTS[s]:STARTS[s] + LENS[s]]
        # token-major skip tiles with proj bias folded in
        skip_tm = np.zeros((128, N_QT * 128), f)
        for j in range(N_QT):
            off = j * KT
            tsz = min(KT, TQ - off)
            skip_tm[0:tsz, 128 * j:128 * j + 128] = sk[:, off:off + tsz].T + proj_b_eff[None, :]
        m = {"xq": xq, "xk": kfm.astype(bf), "xv": vfm.astype(bf),
             "xskip": skip_tm}
        m.update(consts)
        in_maps.append(m)
    return in_maps, flags


_CACHE = {}


def kernel(**inputs):
    f = np.float32
    in_maps, flags = _make_in_maps(inputs)

    key = tuple(sorted(flags.items()))
    if key not in _CACHE:
        _CACHE[key] = build_program(flags)
    nc = _CACHE[key]

    _install_compile_patch()
    res = bass_utils.run_bass_kernel_spmd(nc, in_maps, core_ids=list(range(8)))

    full = np.zeros((B, 128, NQ_FULL), f)
    for c in range(8):
        b, s = c // 4, c % 4
        full[b][:, STARTS[s]:STARTS[s] + LENS[s]] = res.results[c]["y"][:, :LENS[s]]
    return np.ascontiguousarray(
        full.reshape(B, 128, N, M).transpose(0, 2, 1, 3))
